# revision 8
# baseline (speedup 1.0000x reference)
"""Trainium2 Bass kernel for the CustomAutoencoder problem.

7-layer MLP autoencoder over x[8192, 4096], data-parallel over the batch
axis across 8 NeuronCores (1024 rows/core), weights replicated.

The whole network runs in the transposed layout (features on partitions,
batch on the free axis) end to end, including the final layer, so no PE
transposes are needed anywhere: the host hands each core x already
transposed, and the output leaves the device transposed ([S, B] per
core) and is flipped back on the host during the gather.

Matmul dtypes are fp8e4m3 with DoubleRow (2x PE throughput) wherever the
contraction is >= 256 deep (L1, L2, L4, L6, L7); the two tiny-K layers
(L3: K=10, L5: K=32) run bf16. Activation scales are folded into the
prepacked weights on the host so everything stays inside fp8's normal
range (h1 x16, h2/h3 x32, d1 x4096, d2 x8*4096; Wd2/Wd3 carry x8), and
each layer's PSUM evacuation removes whatever scale the next layer does
not want via the activation's scale/bias operands.

Per-core dataflow:
  L1: h1T = relu(M1.T @ xT)    M1 = 16*W1*C1      fp8 DR, K=4096
  L2: h2T = relu(M2.T @ h1T)   M2 = 2*W2*C2       fp8 DR, K=256 (padded)
  L3: h3T = relu(W3.T @ h2T)                      bf16,   K=10
  L4: zT  = relu(W4.T @ h3T)/32                   fp8 DR, K=1024
  L5: d1T = relu(4096*Wd1.T @ zT)                 bf16,   K=32
  L6: d2T = relu(8*Wd2.T @ d1T)                   fp8 DR, K=1024
  L7: outT = sigmoid((8*Wd3).T @ d2T / 2^18 + bd3) fp8 DR, K=2048
"""

import sys

if "/opt/trn_rl_repo" not in sys.path:
    sys.path.insert(0, "/opt/trn_rl_repo")

import numpy as np
import ml_dtypes

B_FULL, S, H1, H2, D4, LAT, DD1, DD2 = 8192, 4096, 196, 10, 1024, 32, 1024, 2048
N_CORES = 8
B = B_FULL // N_CORES          # 1024 rows per core
P = 128                        # partitions
NT = 512                       # matmul free-dim tile (one PSUM bank of fp32)
NK1 = S // P                   # 32 K-chunks for layer 1
NF7 = S // P                   # 32 M-chunks for layer 7

# activation scales (folded into weights host-side so fp8 operands stay
# in the normal range; removed on-device via activation scale operands)
S1, S2, S3 = 16.0, 32.0, 32.0  # h1, h2, h3
SC = 4096.0                    # d1 (matches fp8 subnormal floor of d1~1e-4)
S6 = 8.0                       # extra on Wd2 -> d2 carries S6*SC
S7 = 8.0                       # extra on Wd3 (raw Wd3 ~0.03 is near the
                               # fp8 min-normal; x8 keeps it normal)

F8 = ml_dtypes.float8_e4m3
BF16 = ml_dtypes.bfloat16

_NC_CACHE = {}
TRACE = False  # set by test.py to capture an NTFF profile of the run


def build_nc():
    import concourse.bacc as bacc
    import concourse.mybir as mybir
    import concourse.tile as tile
    from concourse.masks import make_identity

    f32 = mybir.dt.float32
    bf16 = mybir.dt.bfloat16
    f8 = mybir.dt.float8e4
    AF = mybir.ActivationFunctionType
    DR = mybir.MatmulPerfMode.DoubleRow
    ALU = mybir.AluOpType

    nc = bacc.Bacc("TRN2", target_bir_lowering=False, debug=False,
                   num_devices=N_CORES)

    # ---- DRAM I/O: everything arrives host-prepacked in its on-chip
    # layout ((ko p) m -> p (ko m) for the DR operands), already cast and
    # scale-folded, so every load is a fat contiguous-per-partition DMA.
    x_d = nc.dram_tensor("xq", [S, B], f8, kind="ExternalInput")
    m1_d = nc.dram_tensor("m1p", [P, NK1 * 2 * P], f8,
                          kind="ExternalInput")
    m2_d = nc.dram_tensor("m2p", [P, 2 * 32], f8, kind="ExternalInput")
    w3_d = nc.dram_tensor("w3p", [H2, D4], bf16, kind="ExternalInput")
    w4_d = nc.dram_tensor("w4p", [P, (D4 // P) * LAT], f8,
                          kind="ExternalInput")
    wd1_d = nc.dram_tensor("wd1p", [LAT, DD1], bf16, kind="ExternalInput")
    wd2_d = nc.dram_tensor("wd2p", [P, (DD1 // P) * DD2], f8,
                           kind="ExternalInput")
    wd3_d = nc.dram_tensor("wd3p", [P, (DD2 // P) * S], f8,
                           kind="ExternalInput")
    bias_d = nc.dram_tensor("biasp", [P, 68], f32, kind="ExternalInput")
    out_d = nc.dram_tensor("outT", [S, B], bf16, kind="ExternalOutput")

    with tile.TileContext(nc) as tc:
        with (
            tc.tile_pool(name="const", bufs=1) as cpool,
            tc.tile_pool(name="acts", bufs=1) as apool,
            tc.tile_pool(name="outp", bufs=4) as opool,
        ):
            ident = cpool.tile([P, P], bf16)
            make_identity(nc, ident)

            # ---------------- SBUF residency ----------------
            bias_sb = cpool.tile([P, 68], f32)
            m2_sb = cpool.tile([P, 2, 32], f8)
            w3_sb = cpool.tile([H2, D4], bf16)
            w4_sb = cpool.tile([P, D4 // P, LAT], f8)
            wd1_sb = cpool.tile([LAT, DD1], bf16)

            m1_sb = apool.tile([P, NK1, 2 * P], f8)
            xT = apool.tile([P, NK1, B], f8)
            wd2_sb = apool.tile([P, DD1 // P, DD2], f8)
            wd3_sb = apool.tile([P, DD2 // P, S], f8)

            h1T = apool.tile([P, 2, B], f8)
            h2T = apool.tile([H2, B], bf16)
            h3T = apool.tile([P, D4 // P, B], f8)
            zT = apool.tile([LAT, B], bf16)
            d1T = apool.tile([P, DD1 // P, B], f8)
            d2T = apool.tile([P, DD2 // P, B], f8)

            # bias column map (host-packed):
            #   0:2 S1*b1 | 2:10 S3*b3 | 10:18 SC*bd1 | 18:34 S6*SC*bd2
            #   34 S2*b2 | 35 b4 | 36:68 bd3 (raw)
            b1c, b3c, bd1c, bd2c, b2c, b4c, bd3c = 0, 2, 10, 18, 34, 35, 36

            # ---------------- DMA issue order ----------------
            # m1 + x first (layer 1's critical path), then the mid-chain
            # weights in consumption order, Wd3 last. x in 4-chunk blocks
            # so L1 can start as soon as the first pair lands without
            # paying 32 descriptor-gen issue slots on the sync queue.
            XG = 4                      # ko chunks per x DMA
            MW = 2 * P
            for g in range(NK1 // XG):
                if g % 2 == 0:          # m1 quarter ahead of its x blocks
                    q = g // 2
                    nc.sync.dma_start(
                        m1_sb[:, q * 8 : (q + 1) * 8, :],
                        m1_d[:, q * 8 * MW : (q + 1) * 8 * MW].rearrange(
                            "p (ko m) -> p ko m", m=MW),
                    )
                nc.sync.dma_start(
                    xT[:, g * XG : (g + 1) * XG, :],
                    x_d[g * XG * P : (g + 1) * XG * P, :].rearrange(
                        "(ko p) b -> p ko b", p=P),
                )
            nc.sync.dma_start(bias_sb[:], bias_d[:])
            nc.sync.dma_start(m2_sb[:], m2_d[:].rearrange(
                "p (ko m) -> p ko m", m=32))
            nc.sync.dma_start(w3_sb[:], w3_d[:])
            nc.sync.dma_start(w4_sb[:], w4_d[:].rearrange(
                "p (ko m) -> p ko m", m=LAT))
            nc.sync.dma_start(wd1_sb[:], wd1_d[:])
            for g in range(2):
                h = (DD1 // P) // 2
                nc.sync.dma_start(
                    wd2_sb[:, g * h : (g + 1) * h, :],
                    wd2_d[:, g * h * DD2 : (g + 1) * h * DD2].rearrange(
                        "p (ko m) -> p ko m", m=DD2),
                )
            for g in range(4):
                h = (DD2 // P) // 4
                nc.sync.dma_start(
                    wd3_sb[:, g * h : (g + 1) * h, :],
                    wd3_d[:, g * h * S : (g + 1) * h * S].rearrange(
                        "p (ko m) -> p ko m", m=S),
                )

            # ---------------- stage 1: layer 1 ----------------
            with tc.tile_pool(name="psum_l1", bufs=1, space="PSUM") as pl1:
                # PE warm-up: ~40 back-to-back matmuls lift the HAM clock
                # gate (1.2 -> 2.4 GHz) while the first x chunks stream in.
                warm_ps = pl1.tile([P, P], f32, name="warm")
                for _ in range(40):
                    nc.tensor.matmul(warm_ps[:], ident[:], ident[:],
                                     start=True, stop=True,
                                     skip_group_check=True)

                ps_l1 = [[pl1.tile([P, NT], f32, name=f"l1_{m}_{n}")
                          for n in range(2)] for m in range(2)]
                for j in range(NK1 // 2):      # K pairs (DR: 256/pass)
                    for m in range(2):
                        for n in range(2):
                            nc.tensor.matmul(
                                ps_l1[m][n][:],
                                m1_sb[:, 2 * j : 2 * j + 2,
                                      m * P : (m + 1) * P],
                                xT[:, 2 * j : 2 * j + 2,
                                   n * NT : (n + 1) * NT],
                                start=(j == 0), stop=(j == NK1 // 2 - 1),
                                perf_mode=DR, skip_group_check=True,
                            )
                for m in range(2):
                    mw = P  # pad cols of M1 are zero -> h1T pads land 0
                    bs = bias_sb[0:mw, b1c + m : b1c + m + 1]
                    for n in range(2):
                        ns = slice(n * NT, (n + 1) * NT)
                        if n == 0:
                            nc.scalar.activation(
                                h1T[0:mw, m, ns], ps_l1[m][n][0:mw, :],
                                AF.Relu, bias=bs)
                        else:
                            nc.vector.tensor_scalar(
                                h1T[0:mw, m, ns], ps_l1[m][n][0:mw, :],
                                bs, 0.0, ALU.add, ALU.max)

            # ---------------- layers 2-6 (per 512-wide batch half) ------
            pmm_ctx = tc.tile_pool(name="psum_mm", bufs=6, space="PSUM")
            pmm = pmm_ctx.__enter__()
            NS = [slice(n * NT, (n + 1) * NT) for n in range(B // NT)]
            # L2: K = 196 (one padded DR pass), M = 10 (padded to 32).
            # The two batch halves interleave through every layer so one
            # half's PSUM evacuation hides under the other's matmuls.
            ps2 = []
            for n, ns in enumerate(NS):
                ps = pmm.tile([P, NT], f32, tag="mm")
                ps2.append(ps)
                nc.tensor.matmul(ps[0:32, :], m2_sb[:], h1T[:, :, ns],
                                 start=True, stop=True, perf_mode=DR)
            for n, ns in enumerate(NS):
                nc.scalar.activation(h2T[:, ns], ps2[n][0:H2, :], AF.Relu,
                                     bias=bias_sb[0:H2, b2c : b2c + 1])
            # L3: K = 10, M = 1024, bf16; relu+bias alternates
            # ScalarE/VectorE so evacuation keeps pace with the PE.
            for m in range(D4 // P):
                for n, ns in enumerate(NS):
                    ps = pmm.tile([P, NT], f32, tag="mm")
                    nc.tensor.matmul(ps[:], w3_sb[:, m * P : (m + 1) * P],
                                     h2T[:, ns], start=True, stop=True)
                    if (2 * m + n) % 2 == 0:
                        nc.scalar.activation(h3T[:, m, ns], ps[:], AF.Relu,
                                             bias=bias_sb[:, b3c + m : b3c + m + 1])
                    else:
                        nc.vector.tensor_scalar(h3T[:, m, ns], ps[:],
                                                bias_sb[:, b3c + m : b3c + m + 1],
                                                0.0, ALU.add, ALU.max)
            # L4: K = 1024 (4 DR passes), M = 32; z = relu(ps/S3 + b4)
            ps4 = []
            for _ in range(2):
                ps = pmm.tile([P, NT], f32, tag="mm")
                ps4.append(ps)
            for k in range(D4 // P // 2):
                for n, ns in enumerate(NS):
                    nc.tensor.matmul(ps4[n][0:LAT, :],
                                     w4_sb[:, 2 * k : 2 * k + 2, :],
                                     h3T[:, 2 * k : 2 * k + 2, ns],
                                     start=(k == 0),
                                     stop=(k == D4 // P // 2 - 1),
                                     perf_mode=DR, skip_group_check=True)
            for n, ns in enumerate(NS):
                nc.scalar.activation(zT[:, ns], ps4[n][0:LAT, :], AF.Relu,
                                     bias=bias_sb[0:LAT, b4c : b4c + 1],
                                     scale=1.0 / S3)
            # L5: K = 32, M = 1024, bf16 (wd1 carries SC)
            for m in range(DD1 // P):
                for n, ns in enumerate(NS):
                    ps = pmm.tile([P, NT], f32, tag="mm")
                    nc.tensor.matmul(ps[:], wd1_sb[:, m * P : (m + 1) * P],
                                     zT[:, ns], start=True, stop=True)
                    if (2 * m + n) % 2 == 0:
                        nc.scalar.activation(d1T[:, m, ns], ps[:], AF.Relu,
                                             bias=bias_sb[:, bd1c + m : bd1c + m + 1])
                    else:
                        nc.vector.tensor_scalar(d1T[:, m, ns], ps[:],
                                                bias_sb[:, bd1c + m : bd1c + m + 1],
                                                0.0, ALU.add, ALU.max)
            # L6: K = 1024 (4 DR passes), M = 2048
            for m in range(DD2 // P):
                for n, ns in enumerate(NS):
                    ps = pmm.tile([P, NT], f32, tag="mm")
                    for k in range(DD1 // P // 2):
                        nc.tensor.matmul(ps[:],
                                         wd2_sb[:, 2 * k : 2 * k + 2,
                                                m * P : (m + 1) * P],
                                         d1T[:, 2 * k : 2 * k + 2, ns],
                                         start=(k == 0),
                                         stop=(k == DD1 // P // 2 - 1),
                                         perf_mode=DR)
                    if (2 * m + n) % 2 == 0:
                        nc.scalar.activation(d2T[:, m, ns], ps[:], AF.Relu,
                                             bias=bias_sb[:, bd2c + m : bd2c + m + 1])
                    else:
                        nc.vector.tensor_scalar(d2T[:, m, ns], ps[:],
                                                bias_sb[:, bd2c + m : bd2c + m + 1],
                                                0.0, ALU.add, ALU.max)

            # ---------------- layer 7 (transposed output) ----------------
            # outT[f*128+p, b] = sigmoid(logits/2^18 + bd3[f*128+p]):
            # bd3 is a plain per-partition activation bias here, so no
            # bias matmuls and no extra vector pass.
            for f in range(NF7):
                fs = slice(f * P, (f + 1) * P)
                for n in range(B // NT):
                    ns = slice(n * NT, (n + 1) * NT)
                    ps = pmm.tile([P, NT], f32, tag="mm")
                    for k in range(DD2 // P // 2):
                        nc.tensor.matmul(ps[:],
                                         wd3_sb[:, 2 * k : 2 * k + 2, fs],
                                         d2T[:, 2 * k : 2 * k + 2, ns],
                                         start=(k == 0),
                                         stop=(k == DD2 // P // 2 - 1),
                                         perf_mode=DR)
                    ot = opool.tile([P, NT], bf16, tag="out")
                    nc.scalar.activation(ot[:], ps[:], AF.Sigmoid,
                                         bias=bias_sb[:, bd3c + f : bd3c + f + 1],
                                         scale=1.0 / (S6 * S7 * SC))
                    nc.sync.dma_start(out_d[fs, ns], ot[:])
            pmm_ctx.__exit__(None, None, None)

    nc.compile()
    return nc


def _get_nc():
    if "nc" not in _NC_CACHE:
        _NC_CACHE["nc"] = build_nc()
    return _NC_CACHE["nc"]


def _pack_kom(w, scale, ko):
    """[ko*128, m] fp32 -> [128, ko*m] fp8, K index = ko*128 + p."""
    m = w.shape[1]
    a = (w * scale).reshape(ko, P, m).transpose(1, 0, 2).reshape(P, ko * m)
    return np.ascontiguousarray(a.astype(F8))


def _prep_shared(inp):
    """Host-side prepack of the replicated weights/biases (fp32 numpy)."""
    m1f = np.zeros((S, 2 * P), np.float32)
    m1f[:, :H1] = inp["W1"] * inp["C1"] * S1
    m1 = _pack_kom(m1f, 1.0, NK1)
    m2f = np.zeros((2 * P, 32), np.float32)
    m2f[:H1, :H2] = inp["W2"] * inp["C2"] * (S2 / S1)
    m2 = _pack_kom(m2f, 1.0, 2)
    w3 = np.ascontiguousarray((inp["W3"] * (S3 / S2)).astype(BF16))
    w4 = _pack_kom(inp["W4"], 1.0, D4 // P)
    wd1 = np.ascontiguousarray((inp["Wd1"] * SC).astype(BF16))
    wd2 = _pack_kom(inp["Wd2"], S6, DD1 // P)
    wd3 = _pack_kom(inp["Wd3"], S7, DD2 // P)

    bias = np.zeros((P, 68), np.float32)
    b1p = np.zeros(2 * P, np.float32)
    b1p[:H1] = inp["b1"] * S1
    bias[:, 0:2] = b1p.reshape(2, P).T
    bias[:, 2:10] = (inp["b3"] * S3).reshape(D4 // P, P).T
    bias[:, 10:18] = (inp["bd1"] * SC).reshape(DD1 // P, P).T
    bias[:, 18:34] = (inp["bd2"] * (S6 * SC)).reshape(DD2 // P, P).T
    bias[0:H2, 34] = inp["b2"] * S2
    bias[0:LAT, 35] = inp["b4"]
    bias[:, 36:68] = inp["bd3"].reshape(NF7, P).T
    return {"m1p": m1, "m2p": m2, "w3p": w3, "w4p": w4, "wd1p": wd1,
            "wd2p": wd2, "wd3p": wd3, "biasp": bias}


def kernel(**inputs):
    from concourse.bass_utils import run_bass_kernel_spmd

    nc = _get_nc()
    full = {k: np.asarray(v, dtype=np.float32) for k, v in inputs.items()}
    shared = _prep_shared(full)
    x = full["x"]
    in_maps = []
    for c in range(N_CORES):
        m = dict(shared)
        m["xq"] = np.ascontiguousarray(
            x[c * B : (c + 1) * B].T.astype(F8))
        in_maps.append(m)
    res = run_bass_kernel_spmd(nc, in_maps, core_ids=list(range(N_CORES)),
                               trace=TRACE)
    _NC_CACHE["last_res"] = res
    out = np.empty((B_FULL, S), np.float32)
    for c in range(N_CORES):
        out[c * B : (c + 1) * B] = \
            np.asarray(res.results[c]["outT"]).astype(np.float32).T
    return out


# revision 9
# speedup vs baseline: 1.1419x; 1.1419x over previous
"""Trainium2 Bass kernel for the CustomAutoencoder problem.

7-layer MLP autoencoder over x[8192, 4096], data-parallel over the batch
axis across 8 NeuronCores (1024 rows/core), weights replicated.

The whole network runs in the transposed layout (features on partitions,
batch on the free axis) end to end, including the final layer, so no PE
transposes are needed anywhere: the host hands each core x already
transposed, and the output leaves the device transposed ([S, B] per
core) and is flipped back on the host during the gather.

Matmul dtypes are fp8e4m3 with DoubleRow (2x PE throughput) wherever the
contraction is >= 256 deep (L1, L2, L4, L6, L7); the two tiny-K layers
(L3: K=10, L5: K=32) run bf16. Activation scales are folded into the
prepacked weights on the host so everything stays inside fp8's normal
range (h1 x16, h2/h3 x32, d1 x4096, d2 x8*4096; Wd2/Wd3 carry x8), and
each layer's PSUM evacuation removes whatever scale the next layer does
not want via the activation's scale/bias operands.

Per-core dataflow:
  L1: h1T = relu(M1.T @ xT)    M1 = 16*W1*C1      fp8 DR, K=4096
  L2: h2T = relu(M2.T @ h1T)   M2 = 2*W2*C2       fp8 DR, K=256 (padded)
  L3: h3T = relu(W3.T @ h2T)                      bf16,   K=10
  L4: zT  = relu(W4.T @ h3T)/32                   fp8 DR, K=1024
  L5: d1T = relu(4096*Wd1.T @ zT)                 bf16,   K=32
  L6: d2T = relu(8*Wd2.T @ d1T)                   fp8 DR, K=1024
  L7: outT = sigmoid((8*Wd3).T @ d2T / 2^18 + bd3) fp8 DR, K=2048
"""

import sys

if "/opt/trn_rl_repo" not in sys.path:
    sys.path.insert(0, "/opt/trn_rl_repo")

import numpy as np
import ml_dtypes

B_FULL, S, H1, H2, D4, LAT, DD1, DD2 = 8192, 4096, 196, 10, 1024, 32, 1024, 2048
N_CORES = 8
B = B_FULL // N_CORES          # 1024 rows per core
P = 128                        # partitions
NT = 512                       # matmul free-dim tile (one PSUM bank of fp32)
NK1 = S // P                   # 32 K-chunks for layer 1
NF7 = S // P                   # 32 M-chunks for layer 7

# activation scales (folded into weights host-side so fp8 operands stay
# in the normal range; removed on-device via activation scale operands)
S1, S2, S3 = 16.0, 32.0, 32.0  # h1, h2, h3
SC = 4096.0                    # d1 (matches fp8 subnormal floor of d1~1e-4)
S6 = 8.0                       # extra on Wd2 -> d2 carries S6*SC
S7 = 8.0                       # extra on Wd3 (raw Wd3 ~0.03 is near the
                               # fp8 min-normal; x8 keeps it normal)

F8 = ml_dtypes.float8_e4m3
BF16 = ml_dtypes.bfloat16

_NC_CACHE = {}
TRACE = False  # set by test.py to capture an NTFF profile of the run


def build_nc():
    import concourse.bacc as bacc
    import concourse.mybir as mybir
    import concourse.tile as tile
    from concourse.masks import make_identity

    f32 = mybir.dt.float32
    bf16 = mybir.dt.bfloat16
    f8 = mybir.dt.float8e4
    AF = mybir.ActivationFunctionType
    DR = mybir.MatmulPerfMode.DoubleRow
    ALU = mybir.AluOpType

    nc = bacc.Bacc("TRN2", target_bir_lowering=False, debug=False,
                   num_devices=N_CORES)

    # ---- DRAM I/O: everything arrives host-prepacked in its on-chip
    # layout ((ko p) m -> p (ko m) for the DR operands), already cast and
    # scale-folded, so every load is a fat contiguous-per-partition DMA.
    x_d = nc.dram_tensor("xq", [S, B], f8, kind="ExternalInput")
    m1_d = nc.dram_tensor("m1p", [P, NK1 * 2 * P], f8,
                          kind="ExternalInput")
    m2_d = nc.dram_tensor("m2p", [P, 2 * 32], f8, kind="ExternalInput")
    w3_d = nc.dram_tensor("w3p", [H2, D4], bf16, kind="ExternalInput")
    w4_d = nc.dram_tensor("w4p", [P, (D4 // P) * LAT], f8,
                          kind="ExternalInput")
    wd1_d = nc.dram_tensor("wd1p", [LAT, DD1], bf16, kind="ExternalInput")
    wd2_d = nc.dram_tensor("wd2p", [P, (DD1 // P) * DD2], f8,
                           kind="ExternalInput")
    wd3_d = nc.dram_tensor("wd3p", [P, (DD2 // P) * S], f8,
                           kind="ExternalInput")
    bias_d = nc.dram_tensor("biasp", [P, 68], f32, kind="ExternalInput")
    out_d = nc.dram_tensor("outT", [S, B], bf16, kind="ExternalOutput")

    with tile.TileContext(nc) as tc:
        with (
            tc.tile_pool(name="const", bufs=1) as cpool,
            tc.tile_pool(name="acts", bufs=1) as apool,
            tc.tile_pool(name="outp", bufs=4) as opool,
        ):
            ident = cpool.tile([P, P], bf16)
            make_identity(nc, ident)

            # ---------------- SBUF residency ----------------
            bias_sb = cpool.tile([P, 68], f32)
            m2_sb = cpool.tile([P, 2, 32], f8)
            w3_sb = cpool.tile([H2, D4], bf16)
            w4_sb = cpool.tile([P, D4 // P, LAT], f8)
            wd1_sb = cpool.tile([LAT, DD1], bf16)

            m1_sb = apool.tile([P, NK1, 2 * P], f8)
            xT = apool.tile([P, NK1, B], f8)
            wd2_sb = apool.tile([P, DD1 // P, DD2], f8)
            wd3_sb = apool.tile([P, DD2 // P, S], f8)

            h1T = apool.tile([P, 2, B], f8)
            h2T = apool.tile([H2, B], bf16)
            h3T = apool.tile([P, D4 // P, B], f8)
            zT = apool.tile([LAT, B], bf16)
            d1T = apool.tile([P, DD1 // P, B], f8)
            d2T = apool.tile([P, DD2 // P, B], f8)

            # bias column map (host-packed):
            #   0:2 S1*b1 | 2:10 S3*b3 | 10:18 SC*bd1 | 18:34 S6*SC*bd2
            #   34 S2*b2 | 35 b4 | 36:68 bd3 (raw)
            b1c, b3c, bd1c, bd2c, b2c, b4c, bd3c = 0, 2, 10, 18, 34, 35, 36

            # ---------------- DMA issue order ----------------
            # m1 + x first (layer 1's critical path), then the mid-chain
            # weights in consumption order, Wd3 last. x in 4-chunk blocks
            # so L1 can start as soon as the first pair lands without
            # paying 32 descriptor-gen issue slots on the sync queue.
            XG = 4                      # ko chunks per x DMA
            MW = 2 * P
            for g in range(NK1 // XG):
                if g % 2 == 0:          # m1 quarter ahead of its x blocks
                    q = g // 2
                    nc.sync.dma_start(
                        m1_sb[:, q * 8 : (q + 1) * 8, :],
                        m1_d[:, q * 8 * MW : (q + 1) * 8 * MW].rearrange(
                            "p (ko m) -> p ko m", m=MW),
                    )
                nc.sync.dma_start(
                    xT[:, g * XG : (g + 1) * XG, :],
                    x_d[g * XG * P : (g + 1) * XG * P, :].rearrange(
                        "(ko p) b -> p ko b", p=P),
                )
            nc.sync.dma_start(bias_sb[:], bias_d[:])
            nc.sync.dma_start(m2_sb[:], m2_d[:].rearrange(
                "p (ko m) -> p ko m", m=32))
            nc.sync.dma_start(w3_sb[:], w3_d[:])
            nc.sync.dma_start(w4_sb[:], w4_d[:].rearrange(
                "p (ko m) -> p ko m", m=LAT))
            nc.sync.dma_start(wd1_sb[:], wd1_d[:])
            for g in range(2):
                h = (DD1 // P) // 2
                nc.sync.dma_start(
                    wd2_sb[:, g * h : (g + 1) * h, :],
                    wd2_d[:, g * h * DD2 : (g + 1) * h * DD2].rearrange(
                        "p (ko m) -> p ko m", m=DD2),
                )
            for g in range(4):
                h = (DD2 // P) // 4
                nc.sync.dma_start(
                    wd3_sb[:, g * h : (g + 1) * h, :],
                    wd3_d[:, g * h * S : (g + 1) * h * S].rearrange(
                        "p (ko m) -> p ko m", m=S),
                )

            # ---------------- stage 1: layer 1 ----------------
            with tc.tile_pool(name="psum_l1", bufs=1, space="PSUM") as pl1:
                # PE warm-up: ~40 back-to-back matmuls lift the HAM clock
                # gate (1.2 -> 2.4 GHz) while the first x chunks stream in.
                warm_ps = pl1.tile([P, P], f32, name="warm")
                for _ in range(40):
                    nc.tensor.matmul(warm_ps[:], ident[:], ident[:],
                                     start=True, stop=True,
                                     skip_group_check=True)

                ps_l1 = [[pl1.tile([P, NT], f32, name=f"l1_{m}_{n}")
                          for n in range(2)] for m in range(2)]
                for j in range(NK1 // 2):      # K pairs (DR: 256/pass)
                    for m in range(2):
                        for n in range(2):
                            nc.tensor.matmul(
                                ps_l1[m][n][:],
                                m1_sb[:, 2 * j : 2 * j + 2,
                                      m * P : (m + 1) * P],
                                xT[:, 2 * j : 2 * j + 2,
                                   n * NT : (n + 1) * NT],
                                start=(j == 0), stop=(j == NK1 // 2 - 1),
                                perf_mode=DR, skip_group_check=True,
                            )
                for m in range(2):
                    mw = P  # pad cols of M1 are zero -> h1T pads land 0
                    bs = bias_sb[0:mw, b1c + m : b1c + m + 1]
                    for n in range(2):
                        ns = slice(n * NT, (n + 1) * NT)
                        if n == 0:
                            nc.scalar.activation(
                                h1T[0:mw, m, ns], ps_l1[m][n][0:mw, :],
                                AF.Relu, bias=bs)
                        else:
                            nc.vector.tensor_scalar(
                                h1T[0:mw, m, ns], ps_l1[m][n][0:mw, :],
                                bs, 0.0, ALU.add, ALU.max)

            # ---------------- layers 2-6 (per 512-wide batch half) ------
            pmm_ctx = tc.tile_pool(name="psum_mm", bufs=6, space="PSUM")
            pmm = pmm_ctx.__enter__()
            NS = [slice(n * NT, (n + 1) * NT) for n in range(B // NT)]
            # L2: K = 196 (one padded DR pass), M = 10 (padded to 32).
            # The two batch halves interleave through every layer so one
            # half's PSUM evacuation hides under the other's matmuls.
            ps2 = []
            for n, ns in enumerate(NS):
                ps = pmm.tile([P, NT], f32, tag="mm")
                ps2.append(ps)
                nc.tensor.matmul(ps[0:32, :], m2_sb[:], h1T[:, :, ns],
                                 start=True, stop=True, perf_mode=DR)
            for n, ns in enumerate(NS):
                nc.scalar.activation(h2T[:, ns], ps2[n][0:H2, :], AF.Relu,
                                     bias=bias_sb[0:H2, b2c : b2c + 1])
            # L3: K = 10, M = 1024, bf16; relu+bias alternates
            # ScalarE/VectorE so evacuation keeps pace with the PE.
            for m in range(D4 // P):
                for n, ns in enumerate(NS):
                    ps = pmm.tile([P, NT], f32, tag="mm")
                    nc.tensor.matmul(ps[:], w3_sb[:, m * P : (m + 1) * P],
                                     h2T[:, ns], start=True, stop=True)
                    if (2 * m + n) % 2 == 0:
                        nc.scalar.activation(h3T[:, m, ns], ps[:], AF.Relu,
                                             bias=bias_sb[:, b3c + m : b3c + m + 1])
                    else:
                        nc.vector.tensor_scalar(h3T[:, m, ns], ps[:],
                                                bias_sb[:, b3c + m : b3c + m + 1],
                                                0.0, ALU.add, ALU.max)
            # L4: K = 1024 (4 DR passes), M = 32; z = relu(ps/S3 + b4)
            ps4 = []
            for _ in range(2):
                ps = pmm.tile([P, NT], f32, tag="mm")
                ps4.append(ps)
            for k in range(D4 // P // 2):
                for n, ns in enumerate(NS):
                    nc.tensor.matmul(ps4[n][0:LAT, :],
                                     w4_sb[:, 2 * k : 2 * k + 2, :],
                                     h3T[:, 2 * k : 2 * k + 2, ns],
                                     start=(k == 0),
                                     stop=(k == D4 // P // 2 - 1),
                                     perf_mode=DR, skip_group_check=True)
            for n, ns in enumerate(NS):
                nc.scalar.activation(zT[:, ns], ps4[n][0:LAT, :], AF.Relu,
                                     bias=bias_sb[0:LAT, b4c : b4c + 1],
                                     scale=1.0 / S3)
            # L5: K = 32, M = 1024, bf16 (wd1 carries SC)
            for m in range(DD1 // P):
                for n, ns in enumerate(NS):
                    ps = pmm.tile([P, NT], f32, tag="mm")
                    nc.tensor.matmul(ps[:], wd1_sb[:, m * P : (m + 1) * P],
                                     zT[:, ns], start=True, stop=True)
                    if (2 * m + n) % 2 == 0:
                        nc.scalar.activation(d1T[:, m, ns], ps[:], AF.Relu,
                                             bias=bias_sb[:, bd1c + m : bd1c + m + 1])
                    else:
                        nc.vector.tensor_scalar(d1T[:, m, ns], ps[:],
                                                bias_sb[:, bd1c + m : bd1c + m + 1],
                                                0.0, ALU.add, ALU.max)
            # L6: K = 1024 (4 DR passes), M = 2048
            for m in range(DD2 // P):
                for n, ns in enumerate(NS):
                    ps = pmm.tile([P, NT], f32, tag="mm")
                    for k in range(DD1 // P // 2):
                        nc.tensor.matmul(ps[:],
                                         wd2_sb[:, 2 * k : 2 * k + 2,
                                                m * P : (m + 1) * P],
                                         d1T[:, 2 * k : 2 * k + 2, ns],
                                         start=(k == 0),
                                         stop=(k == DD1 // P // 2 - 1),
                                         perf_mode=DR)
                    if (2 * m + n) % 2 == 0:
                        nc.scalar.activation(d2T[:, m, ns], ps[:], AF.Relu,
                                             bias=bias_sb[:, bd2c + m : bd2c + m + 1])
                    else:
                        nc.vector.tensor_scalar(d2T[:, m, ns], ps[:],
                                                bias_sb[:, bd2c + m : bd2c + m + 1],
                                                0.0, ALU.add, ALU.max)

            # ---------------- layer 7 (transposed output) ----------------
            # outT[f*128+p, b] = sigmoid(logits/2^18 + bd3[f*128+p]):
            # bd3 is a plain per-partition activation bias here, so no
            # bias matmuls and no extra vector pass.
            for f in range(NF7):
                fs = slice(f * P, (f + 1) * P)
                for n in range(B // NT):
                    ns = slice(n * NT, (n + 1) * NT)
                    ps = pmm.tile([P, NT], f32, tag="mm")
                    for k in range(DD2 // P // 2):
                        nc.tensor.matmul(ps[:],
                                         wd3_sb[:, 2 * k : 2 * k + 2, fs],
                                         d2T[:, 2 * k : 2 * k + 2, ns],
                                         start=(k == 0),
                                         stop=(k == DD2 // P // 2 - 1),
                                         perf_mode=DR)
                    ot = opool.tile([P, NT], bf16, tag="out")
                    nc.scalar.activation(ot[:], ps[:], AF.Sigmoid,
                                         bias=bias_sb[:, bd3c + f : bd3c + f + 1],
                                         scale=1.0 / (S6 * S7 * SC))
                    nc.gpsimd.dma_start(out_d[fs, ns], ot[:])
            pmm_ctx.__exit__(None, None, None)

    nc.compile()
    return nc


def _get_nc():
    if "nc" not in _NC_CACHE:
        _NC_CACHE["nc"] = build_nc()
    return _NC_CACHE["nc"]


def _pack_kom(w, scale, ko):
    """[ko*128, m] fp32 -> [128, ko*m] fp8, K index = ko*128 + p."""
    m = w.shape[1]
    a = (w * scale).reshape(ko, P, m).transpose(1, 0, 2).reshape(P, ko * m)
    return np.ascontiguousarray(a.astype(F8))


def _prep_shared(inp):
    """Host-side prepack of the replicated weights/biases (fp32 numpy)."""
    m1f = np.zeros((S, 2 * P), np.float32)
    m1f[:, :H1] = inp["W1"] * inp["C1"] * S1
    m1 = _pack_kom(m1f, 1.0, NK1)
    m2f = np.zeros((2 * P, 32), np.float32)
    m2f[:H1, :H2] = inp["W2"] * inp["C2"] * (S2 / S1)
    m2 = _pack_kom(m2f, 1.0, 2)
    w3 = np.ascontiguousarray((inp["W3"] * (S3 / S2)).astype(BF16))
    w4 = _pack_kom(inp["W4"], 1.0, D4 // P)
    wd1 = np.ascontiguousarray((inp["Wd1"] * SC).astype(BF16))
    wd2 = _pack_kom(inp["Wd2"], S6, DD1 // P)
    wd3 = _pack_kom(inp["Wd3"], S7, DD2 // P)

    bias = np.zeros((P, 68), np.float32)
    b1p = np.zeros(2 * P, np.float32)
    b1p[:H1] = inp["b1"] * S1
    bias[:, 0:2] = b1p.reshape(2, P).T
    bias[:, 2:10] = (inp["b3"] * S3).reshape(D4 // P, P).T
    bias[:, 10:18] = (inp["bd1"] * SC).reshape(DD1 // P, P).T
    bias[:, 18:34] = (inp["bd2"] * (S6 * SC)).reshape(DD2 // P, P).T
    bias[0:H2, 34] = inp["b2"] * S2
    bias[0:LAT, 35] = inp["b4"]
    bias[:, 36:68] = inp["bd3"].reshape(NF7, P).T
    return {"m1p": m1, "m2p": m2, "w3p": w3, "w4p": w4, "wd1p": wd1,
            "wd2p": wd2, "wd3p": wd3, "biasp": bias}


def kernel(**inputs):
    from concourse.bass_utils import run_bass_kernel_spmd

    nc = _get_nc()
    full = {k: np.asarray(v, dtype=np.float32) for k, v in inputs.items()}
    shared = _prep_shared(full)
    x = full["x"]
    in_maps = []
    for c in range(N_CORES):
        m = dict(shared)
        m["xq"] = np.ascontiguousarray(
            x[c * B : (c + 1) * B].T.astype(F8))
        in_maps.append(m)
    res = run_bass_kernel_spmd(nc, in_maps, core_ids=list(range(N_CORES)),
                               trace=TRACE)
    _NC_CACHE["last_res"] = res
    out = np.empty((B_FULL, S), np.float32)
    for c in range(N_CORES):
        out[c * B : (c + 1) * B] = \
            np.asarray(res.results[c]["outT"]).astype(np.float32).T
    return out


# revision 10
# speedup vs baseline: 1.1664x; 1.0215x over previous
"""Trainium2 Bass kernel for the CustomAutoencoder problem.

7-layer MLP autoencoder over x[8192, 4096], data-parallel over the batch
axis across 8 NeuronCores (1024 rows/core), weights replicated.

The whole network runs in the transposed layout (features on partitions,
batch on the free axis) end to end, including the final layer, so no PE
transposes are needed anywhere: the host hands each core x already
transposed, and the output leaves the device transposed ([S, B] per
core) and is flipped back on the host during the gather.

Matmul dtypes are fp8e4m3 with DoubleRow (2x PE throughput) wherever the
contraction is >= 256 deep (L1, L2, L4, L6, L7); the two tiny-K layers
(L3: K=10, L5: K=32) run bf16. Activation scales are folded into the
prepacked weights on the host so everything stays inside fp8's normal
range (h1 x16, h2/h3 x32, d1 x4096, d2 x8*4096; Wd2/Wd3 carry x8), and
each layer's PSUM evacuation removes whatever scale the next layer does
not want via the activation's scale/bias operands.

Per-core dataflow:
  L1: h1T = relu(M1.T @ xT)    M1 = 16*W1*C1      fp8 DR, K=4096
  L2: h2T = relu(M2.T @ h1T)   M2 = 2*W2*C2       fp8 DR, K=256 (padded)
  L3: h3T = relu(W3.T @ h2T)                      bf16,   K=10
  L4: zT  = relu(W4.T @ h3T)/32                   fp8 DR, K=1024
  L5: d1T = relu(4096*Wd1.T @ zT)                 bf16,   K=32
  L6: d2T = relu(8*Wd2.T @ d1T)                   fp8 DR, K=1024
  L7: outT = sigmoid((8*Wd3).T @ d2T / 2^18 + bd3) fp8 DR, K=2048
"""

import sys

if "/opt/trn_rl_repo" not in sys.path:
    sys.path.insert(0, "/opt/trn_rl_repo")

import numpy as np
import ml_dtypes

B_FULL, S, H1, H2, D4, LAT, DD1, DD2 = 8192, 4096, 196, 10, 1024, 32, 1024, 2048
N_CORES = 8
B = B_FULL // N_CORES          # 1024 rows per core
P = 128                        # partitions
NT = 512                       # matmul free-dim tile (one PSUM bank of fp32)
NK1 = S // P                   # 32 K-chunks for layer 1
NF7 = S // P                   # 32 M-chunks for layer 7

# activation scales (folded into weights host-side so fp8 operands stay
# in the normal range; removed on-device via activation scale operands)
S1, S2, S3 = 16.0, 32.0, 32.0  # h1, h2, h3
SC = 4096.0                    # d1 (matches fp8 subnormal floor of d1~1e-4)
S6 = 8.0                       # extra on Wd2 -> d2 carries S6*SC
S7 = 8.0                       # extra on Wd3 (raw Wd3 ~0.03 is near the
                               # fp8 min-normal; x8 keeps it normal)

F8 = ml_dtypes.float8_e4m3
BF16 = ml_dtypes.bfloat16

_NC_CACHE = {}
TRACE = False  # set by test.py to capture an NTFF profile of the run


def build_nc():
    import concourse.bacc as bacc
    import concourse.mybir as mybir
    import concourse.tile as tile
    from concourse.masks import make_identity

    f32 = mybir.dt.float32
    bf16 = mybir.dt.bfloat16
    f8 = mybir.dt.float8e4
    AF = mybir.ActivationFunctionType
    DR = mybir.MatmulPerfMode.DoubleRow
    ALU = mybir.AluOpType

    nc = bacc.Bacc("TRN2", target_bir_lowering=False, debug=False,
                   num_devices=N_CORES)

    # ---- DRAM I/O: everything arrives host-prepacked in its on-chip
    # layout ((ko p) m -> p (ko m) for the DR operands), already cast and
    # scale-folded, so every load is a fat contiguous-per-partition DMA.
    x_d = nc.dram_tensor("xq", [S, B], f8, kind="ExternalInput")
    m1_d = nc.dram_tensor("m1p", [P, NK1 * 2 * P], f8,
                          kind="ExternalInput")
    m2_d = nc.dram_tensor("m2p", [P, 2 * 32], f8, kind="ExternalInput")
    w3_d = nc.dram_tensor("w3p", [H2, D4], bf16, kind="ExternalInput")
    w4_d = nc.dram_tensor("w4p", [P, (D4 // P) * LAT], f8,
                          kind="ExternalInput")
    wd1_d = nc.dram_tensor("wd1p", [LAT, DD1], bf16, kind="ExternalInput")
    wd2_d = nc.dram_tensor("wd2p", [P, (DD1 // P) * DD2], f8,
                           kind="ExternalInput")
    wd3_d = nc.dram_tensor("wd3p", [P, (DD2 // P) * S], f8,
                           kind="ExternalInput")
    bias_d = nc.dram_tensor("biasp", [P, 68], f32, kind="ExternalInput")
    out_d = nc.dram_tensor("outT", [S, B], bf16, kind="ExternalOutput")

    with tile.TileContext(nc) as tc:
        with (
            tc.tile_pool(name="const", bufs=1) as cpool,
            tc.tile_pool(name="acts", bufs=1) as apool,
            tc.tile_pool(name="outp", bufs=4) as opool,
        ):
            ident = cpool.tile([P, P], bf16)
            make_identity(nc, ident)

            # ---------------- SBUF residency ----------------
            bias_sb = cpool.tile([P, 68], f32)
            m2_sb = cpool.tile([P, 2, 32], f8)
            w3_sb = cpool.tile([H2, D4], bf16)
            w4_sb = cpool.tile([P, D4 // P, LAT], f8)
            wd1_sb = cpool.tile([LAT, DD1], bf16)

            m1_sb = apool.tile([P, NK1, 2 * P], f8)
            xT = apool.tile([P, NK1, B], f8)
            wd2_sb = apool.tile([P, DD1 // P, DD2], f8)
            wd3_sb = apool.tile([P, DD2 // P, S], f8)

            h1T = apool.tile([P, 2, B], f8)
            h2T = apool.tile([H2, B], bf16)
            h3T = apool.tile([P, D4 // P, B], f8)
            zT = apool.tile([LAT, B], bf16)
            d1T = apool.tile([P, DD1 // P, B], f8)
            d2T = apool.tile([P, DD2 // P, B], f8)

            # bias column map (host-packed):
            #   0:2 S1*b1 | 2:10 S3*b3 | 10:18 SC*bd1 | 18:34 S6*SC*bd2
            #   34 S2*b2 | 35 b4 | 36:68 bd3 (raw)
            b1c, b3c, bd1c, bd2c, b2c, b4c, bd3c = 0, 2, 10, 18, 34, 35, 36

            # ---------------- DMA issue order ----------------
            # m1 + x first (layer 1's critical path), then the mid-chain
            # weights in consumption order, Wd3 last. x in 4-chunk blocks
            # so L1 can start as soon as the first pair lands without
            # paying 32 descriptor-gen issue slots on the sync queue.
            MW = 2 * P
            xblocks = [(0, 2), (2, 2), (4, 4), (8, 4), (12, 4), (16, 4),
                       (20, 4), (24, 4), (28, 4)]
            m1_at = {0: (0, 4), 2: (4, 4), 8: (8, 8), 16: (16, 8),
                     24: (24, 8)}
            for k0, kn in xblocks:
                if k0 in m1_at:         # m1 slice ahead of its x block
                    q0, qn = m1_at[k0]
                    nc.sync.dma_start(
                        m1_sb[:, q0 : q0 + qn, :],
                        m1_d[:, q0 * MW : (q0 + qn) * MW].rearrange(
                            "p (ko m) -> p ko m", m=MW),
                    )
                nc.sync.dma_start(
                    xT[:, k0 : k0 + kn, :],
                    x_d[k0 * P : (k0 + kn) * P, :].rearrange(
                        "(ko p) b -> p ko b", p=P),
                )
            nc.sync.dma_start(bias_sb[:], bias_d[:])
            nc.sync.dma_start(m2_sb[:], m2_d[:].rearrange(
                "p (ko m) -> p ko m", m=32))
            nc.sync.dma_start(w3_sb[:], w3_d[:])
            nc.sync.dma_start(w4_sb[:], w4_d[:].rearrange(
                "p (ko m) -> p ko m", m=LAT))
            nc.sync.dma_start(wd1_sb[:], wd1_d[:])
            for g in range(2):
                h = (DD1 // P) // 2
                nc.sync.dma_start(
                    wd2_sb[:, g * h : (g + 1) * h, :],
                    wd2_d[:, g * h * DD2 : (g + 1) * h * DD2].rearrange(
                        "p (ko m) -> p ko m", m=DD2),
                )
            for g in range(4):
                h = (DD2 // P) // 4
                nc.sync.dma_start(
                    wd3_sb[:, g * h : (g + 1) * h, :],
                    wd3_d[:, g * h * S : (g + 1) * h * S].rearrange(
                        "p (ko m) -> p ko m", m=S),
                )

            # ---------------- stage 1: layer 1 ----------------
            with tc.tile_pool(name="psum_l1", bufs=1, space="PSUM") as pl1:
                # PE warm-up: ~40 back-to-back matmuls lift the HAM clock
                # gate (1.2 -> 2.4 GHz) while the first x chunks stream in.
                warm_ps = pl1.tile([P, P], f32, name="warm")
                for _ in range(40):
                    nc.tensor.matmul(warm_ps[:], ident[:], ident[:],
                                     start=True, stop=True,
                                     skip_group_check=True)

                ps_l1 = [[pl1.tile([P, NT], f32, name=f"l1_{m}_{n}")
                          for n in range(2)] for m in range(2)]
                for j in range(NK1 // 2):      # K pairs (DR: 256/pass)
                    for m in range(2):
                        for n in range(2):
                            nc.tensor.matmul(
                                ps_l1[m][n][:],
                                m1_sb[:, 2 * j : 2 * j + 2,
                                      m * P : (m + 1) * P],
                                xT[:, 2 * j : 2 * j + 2,
                                   n * NT : (n + 1) * NT],
                                start=(j == 0), stop=(j == NK1 // 2 - 1),
                                perf_mode=DR, skip_group_check=True,
                            )
                for m in range(2):
                    mw = P  # pad cols of M1 are zero -> h1T pads land 0
                    bs = bias_sb[0:mw, b1c + m : b1c + m + 1]
                    for n in range(2):
                        ns = slice(n * NT, (n + 1) * NT)
                        if (m + n) % 2 == 0:
                            nc.scalar.activation(
                                h1T[0:mw, m, ns], ps_l1[m][n][0:mw, :],
                                AF.Relu, bias=bs)
                        else:
                            nc.vector.tensor_scalar(
                                h1T[0:mw, m, ns], ps_l1[m][n][0:mw, :],
                                bs, 0.0, ALU.add, ALU.max)

            # ---------------- layers 2-6 (per 512-wide batch half) ------
            pmm_ctx = tc.tile_pool(name="psum_mm", bufs=6, space="PSUM")
            pmm = pmm_ctx.__enter__()
            NS = [slice(n * NT, (n + 1) * NT) for n in range(B // NT)]
            # L2: K = 196 (one padded DR pass), M = 10 (padded to 32).
            # The two batch halves interleave through every layer so one
            # half's PSUM evacuation hides under the other's matmuls.
            ps2 = []
            for n, ns in enumerate(NS):
                ps = pmm.tile([P, NT], f32, tag="mm")
                ps2.append(ps)
                nc.tensor.matmul(ps[0:32, :], m2_sb[:], h1T[:, :, ns],
                                 start=True, stop=True, perf_mode=DR)
            for n, ns in enumerate(NS):
                if n == 0:
                    nc.scalar.activation(h2T[:, ns], ps2[n][0:H2, :],
                                         AF.Relu,
                                         bias=bias_sb[0:H2, b2c : b2c + 1])
                else:
                    nc.vector.tensor_scalar(h2T[:, ns], ps2[n][0:H2, :],
                                            bias_sb[0:H2, b2c : b2c + 1],
                                            0.0, ALU.add, ALU.max)
            # L3: K = 10, M = 1024, bf16; relu+bias alternates
            # ScalarE/VectorE so evacuation keeps pace with the PE.
            for m in range(D4 // P):
                for n, ns in enumerate(NS):
                    ps = pmm.tile([P, NT], f32, tag="mm")
                    nc.tensor.matmul(ps[:], w3_sb[:, m * P : (m + 1) * P],
                                     h2T[:, ns], start=True, stop=True)
                    if (2 * m + n) % 2 == 0:
                        nc.scalar.activation(h3T[:, m, ns], ps[:], AF.Relu,
                                             bias=bias_sb[:, b3c + m : b3c + m + 1])
                    else:
                        nc.vector.tensor_scalar(h3T[:, m, ns], ps[:],
                                                bias_sb[:, b3c + m : b3c + m + 1],
                                                0.0, ALU.add, ALU.max)
            # L4: K = 1024 (4 DR passes), M = 32; z = relu(ps/S3 + b4)
            ps4 = []
            for _ in range(2):
                ps = pmm.tile([P, NT], f32, tag="mm")
                ps4.append(ps)
            for k in range(D4 // P // 2):
                for n, ns in enumerate(NS):
                    nc.tensor.matmul(ps4[n][0:LAT, :],
                                     w4_sb[:, 2 * k : 2 * k + 2, :],
                                     h3T[:, 2 * k : 2 * k + 2, ns],
                                     start=(k == 0),
                                     stop=(k == D4 // P // 2 - 1),
                                     perf_mode=DR, skip_group_check=True)
            for n, ns in enumerate(NS):
                if n == 0:
                    nc.scalar.activation(zT[:, ns], ps4[n][0:LAT, :],
                                         AF.Relu,
                                         bias=bias_sb[0:LAT, b4c : b4c + 1])
                else:
                    nc.vector.tensor_scalar(zT[:, ns], ps4[n][0:LAT, :],
                                            bias_sb[0:LAT, b4c : b4c + 1],
                                            0.0, ALU.add, ALU.max)
            # L5: K = 32, M = 1024, bf16 (wd1 carries SC)
            for m in range(DD1 // P):
                for n, ns in enumerate(NS):
                    ps = pmm.tile([P, NT], f32, tag="mm")
                    nc.tensor.matmul(ps[:], wd1_sb[:, m * P : (m + 1) * P],
                                     zT[:, ns], start=True, stop=True)
                    if (2 * m + n) % 2 == 0:
                        nc.scalar.activation(d1T[:, m, ns], ps[:], AF.Relu,
                                             bias=bias_sb[:, bd1c + m : bd1c + m + 1])
                    else:
                        nc.vector.tensor_scalar(d1T[:, m, ns], ps[:],
                                                bias_sb[:, bd1c + m : bd1c + m + 1],
                                                0.0, ALU.add, ALU.max)
            # L6: K = 1024 (4 DR passes), M = 2048
            for m in range(DD2 // P):
                for n, ns in enumerate(NS):
                    ps = pmm.tile([P, NT], f32, tag="mm")
                    for k in range(DD1 // P // 2):
                        nc.tensor.matmul(ps[:],
                                         wd2_sb[:, 2 * k : 2 * k + 2,
                                                m * P : (m + 1) * P],
                                         d1T[:, 2 * k : 2 * k + 2, ns],
                                         start=(k == 0),
                                         stop=(k == DD1 // P // 2 - 1),
                                         perf_mode=DR)
                    if (2 * m + n) % 2 == 0:
                        nc.scalar.activation(d2T[:, m, ns], ps[:], AF.Relu,
                                             bias=bias_sb[:, bd2c + m : bd2c + m + 1])
                    else:
                        nc.vector.tensor_scalar(d2T[:, m, ns], ps[:],
                                                bias_sb[:, bd2c + m : bd2c + m + 1],
                                                0.0, ALU.add, ALU.max)

            # ---------------- layer 7 (transposed output) ----------------
            # outT[f*128+p, b] = sigmoid(logits/2^18 + bd3[f*128+p]):
            # bd3 is a plain per-partition activation bias here, so no
            # bias matmuls and no extra vector pass.
            for f in range(NF7):
                fs = slice(f * P, (f + 1) * P)
                for n in range(B // NT):
                    ns = slice(n * NT, (n + 1) * NT)
                    ps = pmm.tile([P, NT], f32, tag="mm")
                    for k in range(DD2 // P // 2):
                        nc.tensor.matmul(ps[:],
                                         wd3_sb[:, 2 * k : 2 * k + 2, fs],
                                         d2T[:, 2 * k : 2 * k + 2, ns],
                                         start=(k == 0),
                                         stop=(k == DD2 // P // 2 - 1),
                                         perf_mode=DR)
                    ot = opool.tile([P, NT], bf16, tag="out")
                    nc.scalar.activation(ot[:], ps[:], AF.Sigmoid,
                                         bias=bias_sb[:, bd3c + f : bd3c + f + 1],
                                         scale=1.0 / (S6 * S7 * SC))
                    if f >= NF7 - 4:
                        nc.sync.dma_start(out_d[fs, ns], ot[:])
                    else:
                        nc.gpsimd.dma_start(out_d[fs, ns], ot[:])
            pmm_ctx.__exit__(None, None, None)

    nc.compile()
    return nc


def _get_nc():
    if "nc" not in _NC_CACHE:
        _NC_CACHE["nc"] = build_nc()
    return _NC_CACHE["nc"]


def _pack_kom(w, scale, ko):
    """[ko*128, m] fp32 -> [128, ko*m] fp8, K index = ko*128 + p."""
    m = w.shape[1]
    a = (w * scale).reshape(ko, P, m).transpose(1, 0, 2).reshape(P, ko * m)
    return np.ascontiguousarray(a.astype(F8))


def _prep_shared(inp):
    """Host-side prepack of the replicated weights/biases (fp32 numpy)."""
    m1f = np.zeros((S, 2 * P), np.float32)
    m1f[:, :H1] = inp["W1"] * inp["C1"] * S1
    m1 = _pack_kom(m1f, 1.0, NK1)
    m2f = np.zeros((2 * P, 32), np.float32)
    m2f[:H1, :H2] = inp["W2"] * inp["C2"] * (S2 / S1)
    m2 = _pack_kom(m2f, 1.0, 2)
    w3 = np.ascontiguousarray((inp["W3"] * (S3 / S2)).astype(BF16))
    w4 = _pack_kom(inp["W4"], 1.0, D4 // P)
    wd1 = np.ascontiguousarray((inp["Wd1"] * (SC / S3)).astype(BF16))
    wd2 = _pack_kom(inp["Wd2"], S6, DD1 // P)
    wd3 = _pack_kom(inp["Wd3"], S7, DD2 // P)

    bias = np.zeros((P, 68), np.float32)
    b1p = np.zeros(2 * P, np.float32)
    b1p[:H1] = inp["b1"] * S1
    bias[:, 0:2] = b1p.reshape(2, P).T
    bias[:, 2:10] = (inp["b3"] * S3).reshape(D4 // P, P).T
    bias[:, 10:18] = (inp["bd1"] * SC).reshape(DD1 // P, P).T
    bias[:, 18:34] = (inp["bd2"] * (S6 * SC)).reshape(DD2 // P, P).T
    bias[0:H2, 34] = inp["b2"] * S2
    bias[0:LAT, 35] = inp["b4"] * S3
    bias[:, 36:68] = inp["bd3"].reshape(NF7, P).T
    return {"m1p": m1, "m2p": m2, "w3p": w3, "w4p": w4, "wd1p": wd1,
            "wd2p": wd2, "wd3p": wd3, "biasp": bias}


def kernel(**inputs):
    from concourse.bass_utils import run_bass_kernel_spmd

    nc = _get_nc()
    full = {k: np.asarray(v, dtype=np.float32) for k, v in inputs.items()}
    shared = _prep_shared(full)
    x = full["x"]
    in_maps = []
    for c in range(N_CORES):
        m = dict(shared)
        m["xq"] = np.ascontiguousarray(
            x[c * B : (c + 1) * B].T.astype(F8))
        in_maps.append(m)
    res = run_bass_kernel_spmd(nc, in_maps, core_ids=list(range(N_CORES)),
                               trace=TRACE)
    _NC_CACHE["last_res"] = res
    out = np.empty((B_FULL, S), np.float32)
    for c in range(N_CORES):
        out[c * B : (c + 1) * B] = \
            np.asarray(res.results[c]["outT"]).astype(np.float32).T
    return out
